# revision 1
# baseline (speedup 1.0000x reference)
"""Multi-head attention (B=4, S=2048, D=1024, H=16, Dh=64) on 8 TRN2 NeuronCores.

Sharding: core c -> batch b = c//2, head-group g = c%2 (8 heads, output cols
g*512:(g+1)*512).  Host ships x pre-transposed ([D, S]) and weights in bf16;
each core runs attention for its (batch, 8 heads) slice; host concatenates the
per-core [2048, 512] outputs.

v2 structure (vs baseline): heads processed in PAIRS with K=64 row-tiled
score matmuls -- kT/qT keep both heads of a pair packed in 128 partitions
(rows 0-63 = even head, 64-127 = odd head); the two scores matmuls of a
pair run CONCURRENTLY in PE row-groups (0,0)/(64,0), halving the score
stream vs the zero-padded K=128 scheme.  sq is tiled at 512 (NT4=4):
per (pair, t4, chunk): 2 concurrent score MMs -> one [128,1024] psum
(h0 cols 0-511 | h1 cols 512-1023) -> ONE exp ACTIVATE for both heads ->
2 AV matmuls (M=65: v plus a ones-column producing softmax denominators)
accumulating into a shared [65,1024] psum pair-accumulator; the v bias
is applied once post-normalization (attention rows sum to 1 exactly),
not per-chunk.
Projections, previous-segment transpose/normalize tails, and output DMAs
are interleaved into the chunk stream as filler under the ScalarE-bound
exp; x is DMA'd in 512-column pieces so the first q/k projections start
~6us in, before the rest of x lands.
"""

import numpy as np
import ml_dtypes
from collections import deque
from contextlib import ExitStack

import concourse.bass as bass
import concourse.bacc as bacc
import concourse.mybir as mybir
import concourse.tile as tile
from concourse.bass_utils import run_bass_kernel_spmd
from concourse.masks import make_identity

F32 = mybir.dt.float32
BF16 = mybir.dt.bfloat16

B, S, D = 4, 2048, 1024
H, DH = 16, 64
N_CORES = 8
HPC = 8          # heads per core
DPC = HPC * DH   # output cols per core = 512
SCALE = 1.0 / 32.0  # 1/sqrt(D)

KD = D // 128    # 8 contraction chunks over d_in
NS = S // 128    # 16 sk chunks
MB = HPC // 2    # 4 head pairs
NT4 = S // 512   # 4 sq tiles of 512

_CACHE = {}


def _build_program():
    nc = bacc.Bacc("TRN2", target_bir_lowering=False, debug=False)

    xt_ext = nc.dram_tensor("xt", [D, S], BF16, kind="ExternalInput").ap()
    wq_ext = nc.dram_tensor("wq", [D, DPC], BF16, kind="ExternalInput").ap()
    wk_ext = nc.dram_tensor("wk", [D, DPC], BF16, kind="ExternalInput").ap()
    wv_ext = nc.dram_tensor("wv", [D, DPC], BF16, kind="ExternalInput").ap()
    bq_ext = nc.dram_tensor("bq", [DPC], F32, kind="ExternalInput").ap()
    bk_ext = nc.dram_tensor("bk", [DPC], F32, kind="ExternalInput").ap()
    bv_ext = nc.dram_tensor("bv", [DPC], F32, kind="ExternalInput").ap()
    out_ext = nc.dram_tensor("out", [S, DPC], F32, kind="ExternalOutput").ap()

    with tile.TileContext(nc, pool_alloc_mode="queue") as tc, ExitStack() as ctx:
        singles = ctx.enter_context(tc.tile_pool(name="singles", bufs=1))

        # --- tiny bias vectors first (they gate the first projection
        # copybacks and would otherwise queue behind MBs of weights) ---
        bq_col = []
        bk_col = []
        for m in range(MB):
            t = singles.tile([128, 1], F32, tag=f"bq{m}", name=f"bq{m}")
            nc.sync.dma_start(
                out=t, in_=bq_ext[m * 128:(m + 1) * 128].rearrange("(p o) -> p o", o=1)
            )
            bq_col.append(t)
            t = singles.tile([128, 1], F32, tag=f"bk{m}", name=f"bk{m}")
            nc.scalar.dma_start(
                out=t, in_=bk_ext[m * 128:(m + 1) * 128].rearrange("(p o) -> p o", o=1)
            )
            bk_col.append(t)
        bv_f32 = singles.tile([1, DPC], F32, tag="bv_f32")
        nc.sync.dma_start(out=bv_f32, in_=bv_ext.rearrange("(o n) -> o n", o=1))
        bv_row = singles.tile([1, DPC], BF16, tag="bv_row")
        nc.vector.tensor_copy(bv_row, bv_f32)

        # --- x in 512-col pieces; piece 0 + wq + wk land first so the pair-0
        # q/k projections can start while the rest of x streams in ---
        xT = [singles.tile([128, S], BF16, tag=f"xT{j}", name=f"xT{j}") for j in range(KD)]
        w_bf = {n: [] for n in ("wq", "wk", "wv")}

        def dma_eng(i):
            # alternate the two hardware DGE rings; a single ring's
            # descriptor dispatch caps DMA throughput
            return nc.sync if i % 2 == 0 else nc.scalar

        def load_x_piece(n):
            for j in range(KD):
                dma_eng(n * KD + j).dma_start(
                    out=xT[j][:, n * 512:(n + 1) * 512],
                    in_=xt_ext[j * 128:(j + 1) * 128, n * 512:(n + 1) * 512],
                )

        load_x_piece(0)
        # wq/wk interleaved per K-chunk: the q00/k00 projection chains
        # consume chunk k only at their k-th matmul, so they pipeline with
        # the load instead of waiting for the full matrix
        for k in range(KD):
            for name, ext in (("wq", wq_ext), ("wk", wk_ext)):
                wb = singles.tile([128, DPC], BF16, tag=f"{name}_bf{k}", name=f"{name}_bf{k}")
                dma_eng(k).dma_start(out=wb, in_=ext[k * 128:(k + 1) * 128, :])
                w_bf[name].append(wb)
        for k in range(KD):
            wb = singles.tile([128, DPC], BF16, tag=f"wv_bf{k}", name=f"wv_bf{k}")
            dma_eng(k).dma_start(out=wb, in_=wv_ext[k * 128:(k + 1) * 128, :])
            w_bf["wv"].append(wb)
        load_x_piece(1)
        load_x_piece(2)
        load_x_piece(3)

        identity = singles.tile([128, 128], BF16, tag="identity")
        make_identity(nc, identity)
        ones_row = singles.tile([1, 128], BF16, tag="ones_row")
        nc.vector.memset(ones_row, 1.0)

        # --- persistent sbuf tensors: pair-packed qT/kT (rows 0-63 even
        # head's projection cols, 64-127 odd head's), v natural + ones col ---
        qT = [singles.tile([128, S], BF16, tag=f"qT{m}", name=f"qT{m}") for m in range(MB)]
        kT = [singles.tile([128, S], BF16, tag=f"kT{m}", name=f"kT{m}") for m in range(MB)]
        vsb = [singles.tile([128, HPC, DH + 1], BF16, tag=f"v{i}", name=f"v{i}") for i in range(NS)]
        out_full = [singles.tile([128, DPC], F32, tag=f"of{i}", name=f"of{i}") for i in range(NS)]

        # --- psum pools: scores 2x[128,1024]f32 (4 banks) + shared
        # accumulator/projection/transpose pool (4 banks) ---
        o_psum = ctx.enter_context(tc.tile_pool(name="o_psum", bufs=2, space="PSUM"))
        s_psum = ctx.enter_context(tc.tile_pool(name="s_psum", bufs=2, space="PSUM"))

        e_pool = ctx.enter_context(tc.tile_pool(name="e_pool", bufs=6))
        cp_sb = ctx.enter_context(tc.tile_pool(name="cp_sb", bufs=4))
        ot_sb = ctx.enter_context(tc.tile_pool(name="ot_sb", bufs=8))

        # bv replicated across partitions (one K=1 matmul): the v bias is
        # applied once in the tail -- attention weights sum to 1 exactly, so
        # out = sum(attn (v'+bv)) = sum(attn v') + bv
        bvp = o_psum.tile([128, DPC], F32, tag="po", name="bvp")
        nc.tensor.matmul(bvp, lhsT=ones_row, rhs=bv_row, start=True, stop=True)
        bv_bc = singles.tile([128, DPC], BF16, tag="bv_bc")
        nc.vector.tensor_copy(bv_bc, bvp)

        # warm the PE clock (HAM) while DMA streams in: each pulse reads the
        # just-arrived first x piece so PE activity spans the load window
        warm = o_psum.tile([128, 512], F32, tag="po", name="warm")
        for j in range(KD):
            for i in range(3):
                nc.tensor.matmul(
                    warm, lhsT=identity, rhs=xT[j][:, 0:512], start=True, stop=True
                )

        def gen_q_proj(m, n):
            """q projection for pair m, seq cols n*512:(n+1)*512."""
            sl = slice(n * 512, (n + 1) * 512)
            ps = o_psum.tile([128, 512], F32, tag="po", name=f"ppq{m}_{n}")
            for k in range(KD):
                nc.tensor.matmul(
                    ps,
                    lhsT=w_bf["wq"][k][:, m * 128:(m + 1) * 128],
                    rhs=xT[k][:, sl],
                    start=(k == 0),
                    stop=(k == KD - 1),
                )
                if k % 2 == 1:
                    yield
            nc.vector.tensor_scalar_add(qT[m][:, sl], ps, bq_col[m])

        def gen_k_proj(m, n):
            sl = slice(n * 512, (n + 1) * 512)
            ps = o_psum.tile([128, 512], F32, tag="po", name=f"ppk{m}_{n}")
            for k in range(KD):
                nc.tensor.matmul(
                    ps,
                    lhsT=w_bf["wk"][k][:, m * 128:(m + 1) * 128],
                    rhs=xT[k][:, sl],
                    start=(k == 0),
                    stop=(k == KD - 1),
                )
                if k % 2 == 1:
                    yield
            nc.vector.tensor_scalar_add(kT[m][:, sl], ps, bk_col[m])

        def gen_v_proj(i):
            ps = o_psum.tile([128, 512], F32, tag="po", name=f"vp{i}")
            for k in range(KD):
                nc.tensor.matmul(
                    ps,
                    lhsT=xT[k][:, i * 128:(i + 1) * 128],
                    rhs=w_bf["wv"][k],
                    start=(k == 0),
                    stop=(k == KD - 1),
                )
                if k % 2 == 1:
                    yield
            nc.vector.tensor_copy(
                vsb[i][:, :, 0:DH], ps.rearrange("p (h d) -> p h d", h=HPC)
            )
            nc.vector.memset(vsb[i][:, :, DH:DH + 1], 1.0)

        def emit_out_dma(i, tail=False):
            # sync ring during the attention stream (the scalar ring's
            # trigger would steal ~600ns of ScalarE from the exp stream);
            # alternate rings for the final drain when ScalarE is idle
            eng = nc.scalar if (tail and i % 2) else nc.sync
            eng.dma_start(out=out_ext[i * 128:(i + 1) * 128, :], in_=out_full[i])

        def emit_exp(e, psc):
            # raw InstActivation with immediate bias/scale: skips the
            # per-partition bias-AP read the bass helper forces for Exp
            imm = lambda v: mybir.ImmediateValue(dtype=mybir.dt.float32, value=v)
            return nc.scalar.add_instruction(
                mybir.InstActivation(
                    name=nc.get_next_instruction_name(),
                    func=mybir.ActivationFunctionType.Exp,
                    ins=[nc.scalar.lower_ap(psc), imm(0.0), imm(SCALE), imm(0.0)],
                    outs=[nc.scalar.lower_ap(e)],
                )
            )

        def gen_tail(m, t4, cp):
            """Transpose/normalize pieces for segment (m, t4), reading the
            sbuf copy cp ([65,1024]: h0 | h1); one quantum per yield."""
            for c2 in range(4):
                for j in range(2):
                    pt = o_psum.tile([128, 65], BF16, tag="po", name=f"pt{m}_{t4}_{c2}_{j}")
                    nc.tensor.transpose(
                        pt,
                        cp[:, j * 512 + c2 * 128:j * 512 + (c2 + 1) * 128],
                        identity[0:65, 0:65],
                    )
                    ot = ot_sb.tile([128, 65], BF16, tag="ot", name=f"ot{m}_{t4}_{c2}_{j}")
                    nc.vector.tensor_copy(ot, pt)
                    rc = ot_sb.tile([128, 1], F32, tag="rc", name=f"rc{m}_{t4}_{c2}_{j}")
                    nc.vector.reciprocal(rc, ot[:, DH:DH + 1])
                    nc.vector.scalar_tensor_tensor(
                        out_full[t4 * 4 + c2][:, (2 * m + j) * DH:(2 * m + j + 1) * DH],
                        ot[:, 0:DH],
                        rc,
                        bv_bc[:, (2 * m + j) * DH:(2 * m + j + 1) * DH],
                        mybir.AluOpType.mult,
                        mybir.AluOpType.add,
                    )
                    if m == MB - 1 and j == 1:
                        # slabs 12-15 drain after the chunk stream ends,
                        # when both DMA rings are free
                        emit_out_dma(t4 * 4 + c2, tail=(t4 * 4 + c2 >= 12))
                    yield
            # drain fully when used as a finisher
            return

        # --- filler machinery: a deque of quanta (generator steps) drained
        # proportionally through each 16-chunk segment.  Quanta of one
        # generator stay contiguous so at most one transient o_psum tile
        # overlaps the live pair-accumulator (pool bufs=2). ---
        filler = deque()
        tails = []
        seg_snapshot = 0

        def push_gen(g, quanta):
            for _ in range(quanta):
                filler.append(g.__next__)
            filler.append(lambda gg=g: deque(gg, maxlen=0))  # finish off

        def drain_frac(c, half=2):
            # by chunk c (0-based), consume (c+1)/16 of the segment snapshot;
            # half=0 stops at the midpoint of this chunk's share (called
            # before the scores matmuls; the rest drains behind the exp)
            want = (seg_snapshot * (c + 1) + NS - 1) // NS
            if half == 0:
                prev = (seg_snapshot * c + NS - 1) // NS
                want = (prev + want + 1) // 2
            want = min(want, seg_snapshot)
            consumed = seg_snapshot - len(filler)
            while consumed < want and filler:
                fn = filler.popleft()
                try:
                    fn()
                except StopIteration:
                    pass
                consumed += 1

        # prelude: only what gates the first chunk's scores/AV runs inline
        # (q00/k00/v0/v1); later k groups and v chunks are emitted inside
        # segment (0,0) just ahead of their consumers, so the exp stream
        # starts ~30us earlier
        for _ in gen_q_proj(0, 0):
            pass
        for _ in gen_k_proj(0, 0):
            pass

        # filler distribution: kT group n of pair m is consumed from chunk
        # 4n of every (m, *) segment; qT group t4 from segment (m, t4) on.
        # Spread each pair's projections across the three preceding segments
        # so no segment's filler exceeds the TensorE slack under the
        # ScalarE-bound exp stream.
        held = [None]
        for m in range(MB):
            for t4 in range(NT4):
                if t4 == 0:
                    if m == 0:
                        push_gen(gen_k_proj(0, 1), 4)
                        push_gen(gen_k_proj(0, 2), 4)
                    push_gen(gen_k_proj(m, 3), 4)
                    push_gen(gen_q_proj(m, 1), 4)
                elif t4 == 1:
                    push_gen(gen_q_proj(m, 2), 4)
                    if m + 1 < MB:
                        push_gen(gen_k_proj(m + 1, 1), 4)
                elif t4 == 2:
                    push_gen(gen_q_proj(m, 3), 4)
                    if m + 1 < MB:
                        push_gen(gen_k_proj(m + 1, 2), 4)
                else:
                    if m + 1 < MB:
                        push_gen(gen_q_proj(m + 1, 0), 4)
                        push_gen(gen_k_proj(m + 1, 0), 4)
                if tails and (t4 > 0 or m > 0):
                    push_gen(tails.pop(0), 8)
                    if tails and m == MB - 1:
                        # last pair has no next-pair projections to fill its
                        # slack: drain a second backlogged tail per segment
                        # so little remains after the chunk stream ends
                        push_gen(tails.pop(0), 8)
                seg_snapshot = len(filler)

                po = o_psum.tile([128, 1024], F32, tag="po", name=f"po{m}_{t4}")
                sq = slice(t4 * 512, (t4 + 1) * 512)

                def emit_av(c, e, po=po, m=m):
                    nc.tensor.matmul(
                        po[0:65, 0:512],
                        lhsT=vsb[c][:, 2 * m, :],
                        rhs=e[:, 0:512],
                        start=(c == 0), stop=(c == NS - 1),
                    )
                    nc.tensor.matmul(
                        po[0:65, 512:1024],
                        lhsT=vsb[c][:, 2 * m + 1, :],
                        rhs=e[:, 512:1024],
                        start=(c == 0), stop=(c == NS - 1),
                    )

                def finish_segment(po=po, m=m, t4=t4):
                    cp = cp_sb.tile([65, 1024], BF16, tag="cp", name=f"cp{m}_{t4}")
                    nc.vector.tensor_copy(cp, po[0:65, :])
                    tails.append(gen_tail(m, t4, cp))

                for c in range(NS):
                    drain_frac(c, half=0)
                    psc = s_psum.tile([128, 1024], F32, tag="psc", name=f"ps{m}_{t4}_{c}")
                    # K=64 score matmuls in PE row-groups (0,*)/(64,*): each
                    # A/B pair executes concurrently (disjoint row groups),
                    # and splitting sq in half lets the second pair's fill
                    # overlap the first pair's drain
                    for q4 in range(2):
                        sq4 = slice(t4 * 512 + q4 * 256, t4 * 512 + (q4 + 1) * 256)
                        nc.tensor.matmul(
                            psc[:, q4 * 256:(q4 + 1) * 256],
                            lhsT=kT[m][0:64, c * 128:(c + 1) * 128],
                            rhs=qT[m][0:64, sq4],
                            start=True, stop=True,
                        )
                        nc.tensor.matmul(
                            psc[:, 512 + q4 * 256:512 + (q4 + 1) * 256],
                            lhsT=kT[m][64:128, c * 128:(c + 1) * 128],
                            rhs=qT[m][64:128, sq4],
                            start=True, stop=True,
                        )
                    e = e_pool.tile([128, 1024], BF16, tag="e", name=f"e{m}_{t4}_{c}")
                    emit_exp(e, psc)
                    if c == 0 and held[0] is not None:
                        # previous segment's peeled last AV pair + its
                        # accumulator copy-out run here, AFTER this
                        # segment's first scores/exp are in flight -- the
                        # exp stream never waits behind them
                        held[0]()
                        held[0] = None
                    if m == 0 and t4 == 0 and c + 2 < NS:
                        # v chunks land behind this chunk's exp, two ahead
                        # of their AV (v0-v2 at chunk 0: the first AVs
                        # dep-stall briefly while the exp stream runs on)
                        vs = range(0, 3) if c == 0 else [c + 2]
                        for i in vs:
                            for _ in gen_v_proj(i):
                                pass
                    drain_frac(c)
                    if c < NS - 1:
                        emit_av(c, e)
                    else:
                        def _held(e=e, emit_av=emit_av, finish_segment=finish_segment):
                            emit_av(NS - 1, e)
                            finish_segment()
                        held[0] = _held

        if held[0] is not None:
            held[0]()
            held[0] = None
        # final tails: anything not yet drained (incl. last output DMAs)
        while tails:
            for _ in tails.pop(0):
                pass

    nc.compile()
    return nc


def _get_program():
    if "nc" not in _CACHE:
        _CACHE["nc"] = _build_program()
    return _CACHE["nc"]


def kernel(x, Wq, bq, Wk, bk, Wv, bv, _trace=False):
    bf = ml_dtypes.bfloat16
    x = np.asarray(x, dtype=np.float32)
    Wq = np.asarray(Wq, dtype=np.float32)
    Wk = np.asarray(Wk, dtype=np.float32)
    Wv = np.asarray(Wv, dtype=np.float32)
    bq = np.ascontiguousarray(np.asarray(bq, dtype=np.float32))
    bk = np.ascontiguousarray(np.asarray(bk, dtype=np.float32))
    bv = np.ascontiguousarray(np.asarray(bv, dtype=np.float32))

    nc = _get_program()

    in_maps = []
    for c in range(N_CORES):
        b, g = c // 2, c % 2
        cols = slice(g * DPC, (g + 1) * DPC)
        in_maps.append(
            {
                "xt": np.ascontiguousarray(x[b].T.astype(bf)),
                "wq": np.ascontiguousarray(Wq[:, cols].astype(bf)),
                "wk": np.ascontiguousarray(Wk[:, cols].astype(bf)),
                "wv": np.ascontiguousarray(Wv[:, cols].astype(bf)),
                "bq": np.ascontiguousarray(bq[cols]),
                "bk": np.ascontiguousarray(bk[cols]),
                "bv": np.ascontiguousarray(bv[cols]),
            }
        )

    res = run_bass_kernel_spmd(nc, in_maps, core_ids=list(range(N_CORES)), trace=_trace)
    _CACHE["last_results"] = res

    out = np.empty((B, S, D), dtype=np.float32)
    for c in range(N_CORES):
        b, g = c // 2, c % 2
        out[b, :, g * DPC:(g + 1) * DPC] = res.results[c]["out"]
    return out

